# revision 1
# baseline (speedup 1.0000x reference)
"""Causal self-attention kernel for Trainium2, sharded over 8 NeuronCores.

Problem: B=4, T=2048, DIM=1024, H=16 heads, head_dim=64, fp32 I/O.

Sharding: (batch, head-group) pairs -> 8 shards. Core c handles batch
b = c//2 and head group g = c%2 (8 heads each). Each core computes its
q/k/v projections for its head slice, causal flash-style attention, and
a partial o_proj against its head-slice of wo. The host sums the two
partial o_proj outputs per batch (the "all-reduce") while gathering.

Layout strategy (per core):
  - Host pre-transposes x and the weight slices so the contraction dim
    (model dim) lands on SBUF partitions, and casts them to bf16.
  - Scores are computed TRANSPOSED: sT[tk, tq] = k @ q^T, so softmax'd
    probabilities come out with tk on partitions -- exactly the layout
    the attn@v matmul needs as its moving operand (lhsT = v).
  - Softmax skips max-subtraction (scores are O(1) by construction:
    q,k ~ N(0,1), dot/8), exp runs on the scalar engine straight out of
    PSUM, and the denominator is obtained for free by augmenting v with
    a ones column.
  - Causal masking inside diagonal 128-tiles is applied by one extra
    accumulating matmul (identity x (-1e9 strictly-lower-tri mask)).
"""

import numpy as np
import ml_dtypes

import concourse.bass as bass
import concourse.bacc as bacc
import concourse.mybir as mybir
import concourse.tile as tile
from concourse.bass import ds, ts
from concourse.bass_utils import run_bass_kernel_spmd

BF16 = mybir.dt.bfloat16
F32 = mybir.dt.float32

T = 2048
D = 1024
DG = 512          # head-group width (8 heads x 64)
NH = 8            # heads per core
DH = 64
P = 128
NT = T // P       # 16 t-tiles
NKO = D // P      # 8 contraction tiles for projections
NC_CHUNK = 1024   # tq chunk width for attention
NCH = T // NC_CHUNK  # 2 chunks

_CACHED = None  # (nc, input names) -- build/trace once per process

MM_N = 512  # max moving free-dim per matmul instruction


def _mm(nc, out, lhsT, rhs, start, stop, out_off=0):
    """matmul out = lhsT.T @ rhs, sliced so no piece crosses a PSUM bank
    boundary. out_off is the column offset of `out` within its psum tile."""
    n = rhs.shape[-1]
    o = 0
    while o < n:
        w = min(n - o, MM_N - ((out_off + o) % MM_N))
        nc.tensor.matmul(
            out[:, ds(o, w)], lhsT=lhsT, rhs=rhs[:, ds(o, w)],
            start=start, stop=stop,
        )
        o += w


def _build_kernel():
    nc = bacc.Bacc("TRN2", target_bir_lowering=False, debug=False)

    xT_d = nc.dram_tensor("xT", [D, T], BF16, kind="ExternalInput").ap()
    wqT_d = nc.dram_tensor("wqT", [D, DG], BF16, kind="ExternalInput").ap()
    wkT_d = nc.dram_tensor("wkT", [D, DG], BF16, kind="ExternalInput").ap()
    wvT_d = nc.dram_tensor("wvT", [D, DG], BF16, kind="ExternalInput").ap()
    woT_d = nc.dram_tensor("woT", [DG, D], BF16, kind="ExternalInput").ap()
    y_d = nc.dram_tensor("y", [T, D], F32, kind="ExternalOutput").ap()

    with tile.TileContext(nc) as tc:
        with (
            tc.tile_pool(name="const", bufs=1) as const,
            tc.tile_pool(name="sb", bufs=1) as sb,
            tc.tile_pool(name="work", bufs=4) as work,
            tc.tile_pool(name="wnorm", bufs=2) as wnorm,
            tc.tile_pool(name="stgp", bufs=2) as stgp,
            tc.tile_pool(name="ps", bufs=2, space="PSUM") as psp,
            tc.tile_pool(name="av", bufs=2, space="PSUM") as avp,
        ):
            # ---- constants ----
            # multiplicative causal mask for diag tiles: 1 where tq >= tk
            mskb = const.tile([P, P], BF16, tag="mskb")
            nc.gpsimd.memset(mskb, 1.0)
            nc.gpsimd.affine_select(
                out=mskb, in_=mskb,
                compare_op=mybir.AluOpType.is_ge,
                fill=0.0, base=0,
                pattern=[[1, P]], channel_multiplier=-1,
            )

            # ---- persistent SBUF tensors ----
            XT = sb.tile([P, NKO, T], BF16, tag="XT")
            WQT = sb.tile([P, NKO, DG], BF16, tag="WQT")
            WKT = sb.tile([P, NKO, DG], BF16, tag="WKT")
            WVT = sb.tile([P, NKO, DG], BF16, tag="WVT")
            WOT = sb.tile([P, DG // P, D], BF16, tag="WOT")
            QT = sb.tile([P, DG // P, T], BF16, tag="QT")
            KT = sb.tile([P, DG // P, T], BF16, tag="KT")
            VA = sb.tile([P, NT, NH, DH + 1], BF16, tag="VA")
            OGT = sb.tile([P, DG // P, T], BF16, tag="OGT")

            # ---- input DMAs (chunked across queues) ----
            xr = xT_d.rearrange("(ko p) t -> p ko t", p=P)
            for k in range(NKO):
                nc.sync.dma_start(XT[:, k, :], xr[:, k, :])
            for wsb, wd in ((WQT, wqT_d), (WKT, wkT_d), (WVT, wvT_d)):
                wr = wd.rearrange("(ko p) n -> p ko n", p=P)
                for k in range(NKO):
                    nc.sync.dma_start(wsb[:, k, :], wr[:, k, :])
            wor = woT_d.rearrange("(jo p) n -> p jo n", p=P)
            for j in range(DG // P):
                nc.sync.dma_start(WOT[:, j, :], wor[:, j, :])

            # v_aug ones column
            nc.gpsimd.memset(VA[:, :, :, DH], 1.0)

            # ---- projections ----
            # qT/kT: out[dg, t] with dg on partitions (4 tiles of 128)
            for wsb, dst in ((WQT, QT), (WKT, KT)):
                for dg in range(DG // P):
                    for c in range(NCH):
                        ps = psp.tile([P, NC_CHUNK], F32, tag="s")
                        for k in range(NKO):
                            _mm(
                                nc, ps,
                                lhsT=wsb[:, k, ts(dg, P)],
                                rhs=XT[:, k, ds(c * NC_CHUNK, NC_CHUNK)],
                                start=(k == 0), stop=(k == NKO - 1),
                            )
                        nc.vector.tensor_copy(dst[:, dg, ds(c * NC_CHUNK, NC_CHUNK)], ps)
            # v: natural [t, dg] layout, written per-head into VA
            for tt in range(NT):
                ps = psp.tile([P, DG], F32, tag="s")
                for k in range(NKO):
                    nc.tensor.matmul(
                        ps,
                        lhsT=XT[:, k, ts(tt, P)],
                        rhs=WVT[:, k, :],
                        start=(k == 0), stop=(k == NKO - 1),
                    )
                nc.vector.tensor_copy(
                    VA[:, tt, :, 0:DH],
                    ps.rearrange("p (h d) -> p h d", h=NH),
                )

            # ---- attention (head pairs interleaved, per tq chunk) ----
            # Paired heads live at partitions 0-63 / 64-127 of the same
            # QT/KT p-tile, so their score matmuls use disjoint PE row
            # groups (concurrent) and the pair keeps the PE fed while the
            # scalar engine runs exp for the other head.
            def attn_scores_pair(pt, c, j):
                """scores for both heads of a pair, interleaved A/B so
                adjacent PE matmuls hit disjoint row groups (rows 0-63 vs
                64-127) and execute concurrently in the array. Returns the
                two expT tiles."""
                lo = max(c * NC_CHUNK, j * P)
                w = (c + 1) * NC_CHUNK - lo
                diag = j * P >= c * NC_CHUNK
                psA = psp.tile([P, NC_CHUNK], F32, tag="s")
                psB = psp.tile([P, NC_CHUNK], F32, tag="s")
                o = 0
                while o < w:
                    ww = min(w - o, MM_N)
                    for po, ps in ((0, psA), (DH, psB)):
                        nc.tensor.matmul(
                            ps[:, ds(o, ww)],
                            lhsT=KT[po:po + DH, pt, ts(j, P)],
                            rhs=QT[po:po + DH, pt, ds(lo + o, ww)],
                            start=True, stop=True,
                        )
                    o += ww
                ets = []
                for ps in (psA, psB):
                    et = work.tile([P, NC_CHUNK], BF16, tag="et")
                    nc.scalar.activation(
                        et[:, :w], ps[:, :w],
                        mybir.ActivationFunctionType.Exp,
                        scale=0.125,
                    )
                    if diag:
                        # zero the lower-left of the diagonal 128-block (DVE
                        # is idle; keeps the mask off the busy PE)
                        nc.vector.tensor_mul(et[:, 0:P], et[:, 0:P], mskb)
                    ets.append(et)
                return ets

            def attn_av(h, av, et, c, j):
                # AV accumulate, per psum bank: bank b of this chunk
                # ([512b, 512b+512)) has its last write at tile
                # j == 8c + 4b + 3, which carries stop=True.
                lo = max(c * NC_CHUNK, j * P)
                w = (c + 1) * NC_CHUNK - lo
                s0 = lo - c * NC_CHUNK
                for b in range(NC_CHUNK // MM_N):
                    blo, bhi = b * MM_N, (b + 1) * MM_N
                    plo, phi = max(s0, blo), min(s0 + w, bhi)
                    if plo >= phi:
                        continue
                    nc.tensor.matmul(
                        av[0:DH + 1, ds(plo, phi - plo)],
                        lhsT=VA[:, j, h, :],
                        rhs=et[:, ds(plo - s0, phi - plo)],
                        start=(j == 0),
                        stop=(j == 8 * c + 4 * b + 3),
                    )

            def attn_normalize(av, dst):
                # dst: [DH, NC_CHUNK] slice; scale av rows 0..63 by 1/row64.
                # First copy the psum accumulator to SBUF so the av slot
                # frees immediately (the PE's next chunk j=0 AV waits on it);
                # the whole divide chain then runs off the critical path.
                un = wnorm.tile([DH + 1, NC_CHUNK], F32, tag="un")
                nc.vector.tensor_copy(un, av[0:DH + 1, :])
                # 1/d as exp(-ln d) on ScalarE: d is a positive softmax
                # denominator and the product feeds a bf16 multiply, so ACT
                # table accuracy is plenty; keeps the slow DVE RECIPROCAL
                # (6.5us for a 1-partition row) off the critical path.
                rec = wnorm.tile([1, NC_CHUNK], F32, tag="rec")
                nc.scalar.activation(
                    rec, un[DH:DH + 1, :], mybir.ActivationFunctionType.Ln,
                )
                recb = wnorm.tile([1, NC_CHUNK], BF16, tag="recb")
                nc.scalar.activation(
                    recb, rec, mybir.ActivationFunctionType.Exp, scale=-1.0,
                )
                # broadcast 1/d across partitions on the (idle) GPSIMD so
                # the PE stream rolls straight into the next chunk
                bcb = wnorm.tile([DH, NC_CHUNK], BF16, tag="bcb")
                nc.gpsimd.partition_broadcast(bcb, recb)
                nc.vector.tensor_mul(dst, un[0:DH, :], bcb)

            for hp in range(NH // 2):
                hA, hB = 2 * hp, 2 * hp + 1
                stg = stgp.tile([DH, T], BF16, tag="stg")
                for c in range(NCH):
                    avA = avp.tile([P, NC_CHUNK], F32, tag="av")
                    avB = avp.tile([P, NC_CHUNK], F32, tag="av")
                    jmax = (c + 1) * NC_CHUNK // P - 1
                    for j in range(jmax + 1):
                        etA, etB = attn_scores_pair(hp, c, j)
                        attn_av(hA, avA, etA, c, j)
                        attn_av(hB, avB, etB, c, j)
                    attn_normalize(avA, OGT[0:DH, hp, ds(c * NC_CHUNK, NC_CHUNK)])
                    attn_normalize(avB, stg[:, ds(c * NC_CHUNK, NC_CHUNK)])
                # partition shift 0-63 -> 64-127 via sbuf-to-sbuf DMA
                nc.sync.dma_start(OGT[DH:P, hp, :], stg[:, :])

            # ---- o_proj partial: y[t, o] = sum_j ogT[j, t] * woT[j, o] ----
            for tt in range(NT):
                ps = psp.tile([P, D], F32, tag="s")
                for jt in range(DG // P):
                    _mm(
                        nc, ps,
                        lhsT=OGT[:, jt, ts(tt, P)],
                        rhs=WOT[:, jt, :],
                        start=(jt == 0), stop=(jt == DG // P - 1),
                    )
                ysb = wnorm.tile([P, D], F32, tag="ysb")
                nc.vector.tensor_copy(ysb, ps)
                nc.sync.dma_start(y_d[ts(tt, P), :], ysb)

    # Pin Exp and Ln to the one table set holding both (same 400-piece
    # resolution); otherwise the table-load pass alternates exp_and_others /
    # natural_log, costing a ~1.4us ACT table load per softmax normalize.
    orig = bacc.get_activation_tables
    pref = "natural_log_exp_and_others"

    def tables_ln_exp_combined(arch):
        t = orig(arch)
        if pref in t:
            for name, funcs in t.items():
                if name != pref:
                    funcs.discard(mybir.ActivationFunctionType.Exp)
                    funcs.discard(mybir.ActivationFunctionType.Ln)
        return t

    bacc.get_activation_tables = tables_ln_exp_combined
    try:
        nc.compile()
    finally:
        bacc.get_activation_tables = orig
    return nc


def _get_nc():
    global _CACHED
    if _CACHED is None:
        _CACHED = _build_kernel()
    return _CACHED


def _shard_inputs(x, wq, wk, wv, wo):
    bf = ml_dtypes.bfloat16
    in_maps = []
    for core in range(8):
        b, g = divmod(core, 2)
        gs = slice(g * DG, (g + 1) * DG)
        in_maps.append({
            "xT": np.ascontiguousarray(x[b].T).astype(bf),
            "wqT": np.ascontiguousarray(wq[gs, :].T).astype(bf),
            "wkT": np.ascontiguousarray(wk[gs, :].T).astype(bf),
            "wvT": np.ascontiguousarray(wv[gs, :].T).astype(bf),
            "woT": np.ascontiguousarray(wo[:, gs].T).astype(bf),
        })
    return in_maps


def kernel(x, wq, wk, wv, wo, _trace=False, _trace_cores=None):
    x = np.asarray(x, dtype=np.float32)
    wq = np.asarray(wq, dtype=np.float32)
    wk = np.asarray(wk, dtype=np.float32)
    wv = np.asarray(wv, dtype=np.float32)
    wo = np.asarray(wo, dtype=np.float32)

    nc = _get_nc()
    in_maps = _shard_inputs(x, wq, wk, wv, wo)
    res = run_bass_kernel_spmd(
        nc, in_maps, core_ids=list(range(8)),
        trace=_trace,
        **({"trace_cores": _trace_cores} if _trace_cores else {}),
    )
    B = x.shape[0]
    y = np.zeros((B, T, D), dtype=np.float32)
    for core in range(8):
        b = core // 2
        y[b] += res.results[core]["y"]
    if _trace:
        return y, res
    return y



# revision 7
# speedup vs baseline: 1.3490x; 1.3490x over previous
"""Causal self-attention kernel for Trainium2, sharded over 8 NeuronCores.

Problem: B=4, T=2048, DIM=1024, H=16 heads, head_dim=64, fp32 I/O.

Sharding: (batch, head-group) pairs -> 8 shards. Core c handles batch
b = c//2 and head group g = c%2 (8 heads each). Each core computes its
q/k/v projections for its head slice, causal flash-style attention, and
a partial o_proj against its head-slice of wo. The host sums the two
partial o_proj outputs per batch (the "all-reduce") while gathering.

Schedule strategy (per core): the attention inner loop is ScalarE-bound
(exp runs at 1.2 GHz x 1 elem/lane; the PE needs only ~60% of that
time for scores+AV), so all projection/o_proj matmuls are emitted as a
work queue of half-fills interleaved into the attention j-loop. The
Tile scheduler then keeps the PE dense (no HAM re-throttle) while ACT
streams exps. Heads run one at a time (not in pairs): that halves the
live PSUM set (scores 2x2 banks double-buffered + AV 2 banks) and
leaves 2 banks for the projection queue's accumulators.

Layout (per core):
  - Host pre-transposes x and the weight slices so the contraction dim
    lands on SBUF partitions, and casts to bf16.
  - Scores are computed TRANSPOSED: sT[tk, tq] = k @ q^T, so softmax'd
    probabilities come out with tk on partitions -- the layout the
    attn@v matmul needs as its moving operand (lhsT = v).
  - Softmax skips max-subtraction (scores are O(1) by construction),
    exp runs on ScalarE straight out of PSUM, and the denominator is
    free via a ones column appended to v.
  - 1/denominator runs on the (otherwise idle) DVE via the fast-approx
    Newton reciprocal, broadcast on GpSimd -- no ACT work at all, so
    ScalarE does nothing but exp.
  - Causal masking inside diagonal 128-tiles: one DVE multiply with a
    0/1 lower-triangle mask.
"""

import numpy as np
import ml_dtypes

import concourse.bass as bass
import concourse.bacc as bacc
import concourse.mybir as mybir
import concourse.tile as tile
from concourse.bass import ds, ts
from concourse.bass_utils import run_bass_kernel_spmd

BF16 = mybir.dt.bfloat16
F32 = mybir.dt.float32

T = 2048
D = 1024
DG = 512          # head-group width (8 heads x 64)
NH = 8            # heads per core
DH = 64
P = 128
NT = T // P       # 16 t-tiles
NKO = D // P      # 8 contraction tiles for projections
NC_CHUNK = 1024   # tq chunk width for attention
NCH = T // NC_CHUNK  # 2 chunks

_CACHED = None  # (nc, input names) -- build/trace once per process

MM_N = 512  # max moving free-dim per matmul instruction (one PSUM bank)


def _build_kernel():
    nc = bacc.Bacc("TRN2", target_bir_lowering=False, debug=False)

    xT_d = nc.dram_tensor("xT", [D, T], BF16, kind="ExternalInput").ap()
    wqT_d = nc.dram_tensor("wqT", [D, DG], BF16, kind="ExternalInput").ap()
    wkT_d = nc.dram_tensor("wkT", [D, DG], BF16, kind="ExternalInput").ap()
    wvT_d = nc.dram_tensor("wvT", [D, DG], BF16, kind="ExternalInput").ap()
    woT_d = nc.dram_tensor("woT", [DG, D], BF16, kind="ExternalInput").ap()
    y_d = nc.dram_tensor("y", [T, D], F32, kind="ExternalOutput").ap()

    with tile.TileContext(nc) as tc:
        with (
            tc.tile_pool(name="const", bufs=1) as const,
            tc.tile_pool(name="sb", bufs=1) as sb,
            tc.tile_pool(name="work", bufs=4) as work,
            tc.tile_pool(name="wnorm", bufs=2) as wnorm,
            tc.tile_pool(name="sc", bufs=2, space="PSUM") as scp,
            tc.tile_pool(name="av", bufs=1, space="PSUM") as avp,
            tc.tile_pool(name="pj", bufs=2, space="PSUM") as pjp,
        ):
            # ---- constants ----
            # multiplicative causal mask for diag tiles: 1 where tq >= tk
            mskb = const.tile([P, P], BF16, tag="mskb")
            nc.gpsimd.memset(mskb, 1.0)
            nc.gpsimd.affine_select(
                out=mskb, in_=mskb,
                compare_op=mybir.AluOpType.is_ge,
                fill=0.0, base=0,
                pattern=[[1, P]], channel_multiplier=-1,
            )

            # ---- persistent SBUF tensors ----
            XT = sb.tile([P, NKO, T], BF16, tag="XT")
            WQT = sb.tile([P, NKO, DG], BF16, tag="WQT")
            WKT = sb.tile([P, NKO, DG], BF16, tag="WKT")
            WVT = sb.tile([P, NKO, DG], BF16, tag="WVT")
            WOT = sb.tile([P, DG // P, D], BF16, tag="WOT")
            QT = sb.tile([P, DG // P, T], BF16, tag="QT")
            KT = sb.tile([P, DG // P, T], BF16, tag="KT")
            VA = sb.tile([P, NT, NH, DH + 1], BF16, tag="VA")
            OGT = sb.tile([P, DG // P, T], BF16, tag="OGT")

            # ---- input DMAs: x t-chunk 0 + q/k weights first, so the
            # first projection fills can start ~immediately ----
            xr = xT_d.rearrange("(ko p) t -> p ko t", p=P)
            for k in range(NKO):
                nc.sync.dma_start(XT[:, k, ds(0, MM_N)], xr[:, k, ds(0, MM_N)])
            for wsb, wd in ((WQT, wqT_d), (WKT, wkT_d)):
                wr = wd.rearrange("(ko p) n -> p ko n", p=P)
                for k in range(NKO):
                    nc.sync.dma_start(wsb[:, k, :], wr[:, k, :])
            for tcn in range(1, T // MM_N):
                for k in range(NKO):
                    nc.sync.dma_start(
                        XT[:, k, ds(tcn * MM_N, MM_N)],
                        xr[:, k, ds(tcn * MM_N, MM_N)],
                    )
            wvr = wvT_d.rearrange("(ko p) n -> p ko n", p=P)
            for k in range(NKO):
                nc.sync.dma_start(WVT[:, k, :], wvr[:, k, :])
            wor = woT_d.rearrange("(jo p) n -> p jo n", p=P)
            for j in range(DG // P):
                nc.sync.dma_start(WOT[:, j, :], wor[:, j, :])

            # v_aug ones column
            nc.gpsimd.memset(VA[:, :, :, DH], 1.0)

            # ---- projection work queue (half-fill closures) ----
            # Each fill accumulates 1 PSUM bank over its contraction and
            # is split into two emission halves so PE bursts stay under
            # ~1us and never starve ACT of its next scores tile.
            def make_qk_fill(wsb, dst, dg, tcn):
                st = {}

                def h1():
                    ps = pjp.tile([P, MM_N], F32, tag="pj")
                    st["ps"] = ps
                    for k in range(4):
                        nc.tensor.matmul(
                            ps, lhsT=wsb[:, k, ts(dg, P)],
                            rhs=XT[:, k, ds(tcn * MM_N, MM_N)],
                            start=(k == 0), stop=False,
                        )

                def h2():
                    ps = st["ps"]
                    for k in range(4, NKO):
                        nc.tensor.matmul(
                            ps, lhsT=wsb[:, k, ts(dg, P)],
                            rhs=XT[:, k, ds(tcn * MM_N, MM_N)],
                            start=False, stop=(k == NKO - 1),
                        )
                    nc.vector.tensor_copy(dst[:, dg, ds(tcn * MM_N, MM_N)], ps)

                return [h1, h2]

            def make_v_fill(tt):
                st = {}

                def h1():
                    ps = pjp.tile([P, MM_N], F32, tag="pj")
                    st["ps"] = ps
                    for k in range(4):
                        nc.tensor.matmul(
                            ps, lhsT=XT[:, k, ts(tt, P)], rhs=WVT[:, k, :],
                            start=(k == 0), stop=False,
                        )

                def h2():
                    ps = st["ps"]
                    for k in range(4, NKO):
                        nc.tensor.matmul(
                            ps, lhsT=XT[:, k, ts(tt, P)], rhs=WVT[:, k, :],
                            start=False, stop=(k == NKO - 1),
                        )
                    nc.vector.tensor_copy(
                        VA[:, tt, :, 0:DH],
                        ps.rearrange("p (h d) -> p h d", h=NH),
                    )

                return [h1, h2]

            def make_oproj_fill(tt, half):
                st = {}

                def h1():
                    ps = pjp.tile([P, MM_N], F32, tag="pj")
                    st["ps"] = ps
                    for jt in range(2):
                        nc.tensor.matmul(
                            ps, lhsT=OGT[:, jt, ts(tt, P)],
                            rhs=WOT[:, jt, ds(half * MM_N, MM_N)],
                            start=(jt == 0), stop=False,
                        )

                def h2():
                    ps = st["ps"]
                    for jt in range(2, DG // P):
                        nc.tensor.matmul(
                            ps, lhsT=OGT[:, jt, ts(tt, P)],
                            rhs=WOT[:, jt, ds(half * MM_N, MM_N)],
                            start=False, stop=(jt == DG // P - 1),
                        )
                    ysb = wnorm.tile([P, MM_N], F32, tag="ysb")
                    nc.vector.tensor_copy(ysb, ps)
                    nc.sync.dma_start(y_d[ts(tt, P), ds(half * MM_N, MM_N)], ysb)

                return [h1, h2]

            # Queue order: everything chunk-0 attention needs first (all
            # pairs' q/k for tq 0:1024, v tiles 0..7), then the chunk-1
            # prerequisites. o_proj halves are appended between sweeps.
            # fill_end[key] = queue index at which that tensor region is
            # fully emitted, so the attention loop can pull exactly its
            # prerequisites and otherwise drain at a steady 1 half per j.
            fills = []
            fill_end = {}

            def add(key, halves):
                fills.extend(halves)
                fill_end[key] = len(fills)

            add(("q", 0, 0), make_qk_fill(WQT, QT, 0, 0))
            add(("k", 0, 0), make_qk_fill(WKT, KT, 0, 0))
            add(("q", 0, 1), make_qk_fill(WQT, QT, 0, 1))
            add(("v", 0), make_v_fill(0))
            add(("k", 0, 1), make_qk_fill(WKT, KT, 0, 1))
            add(("v", 1), make_v_fill(1))
            for tt in range(2, 8):
                add(("v", tt), make_v_fill(tt))
            for dg in range(1, 4):
                for tcn in range(2):
                    add(("q", dg, tcn), make_qk_fill(WQT, QT, dg, tcn))
                    add(("k", dg, tcn), make_qk_fill(WKT, KT, dg, tcn))
            for tcn in range(2, 4):
                add(("q", 0, tcn), make_qk_fill(WQT, QT, 0, tcn))
                add(("k", 0, tcn), make_qk_fill(WKT, KT, 0, tcn))
            for tt in range(8, NT):
                add(("v", tt), make_v_fill(tt))
            for dg in range(1, 4):
                for tcn in range(2, 4):
                    add(("q", dg, tcn), make_qk_fill(WQT, QT, dg, tcn))
                    add(("k", dg, tcn), make_qk_fill(WKT, KT, dg, tcn))

            state = {"fi": 0}

            def pop_until(idx):
                while state["fi"] < idx:
                    fills[state["fi"]]()
                    state["fi"] += 1

            def pop(n=1):
                pop_until(min(state["fi"] + n, len(fills)))

            def need(keys):
                pop_until(max(fill_end[k] for k in keys))

            # ---- attention: chunk-major sweep over heads ----
            def attn_head_chunk(h, c):
                hp, sub = divmod(h, 2)
                po = sub * DH
                av = avp.tile([DH + 1, NC_CHUNK], F32, tag="av")
                jmax = (c + 1) * NC_CHUNK // P - 1
                for j in range(jmax + 1):
                    lo = max(c * NC_CHUNK, j * P)
                    w = (c + 1) * NC_CHUNK - lo
                    req = [("q", hp, tcn)
                           for tcn in range(lo // MM_N, 2 * (c + 1))]
                    req += [("k", hp, j * P // MM_N), ("v", j)]
                    need(req)
                    pop(1)
                    ps = scp.tile([P, NC_CHUNK], F32, tag="sc")
                    o = 0
                    while o < w:
                        ww = min(w - o, MM_N - (o % MM_N))
                        nc.tensor.matmul(
                            ps[:, ds(o, ww)],
                            lhsT=KT[po:po + DH, hp, ts(j, P)],
                            rhs=QT[po:po + DH, hp, ds(lo + o, ww)],
                            start=True, stop=True,
                        )
                        o += ww
                    et = work.tile([P, NC_CHUNK], BF16, tag="et")
                    nc.scalar.activation(
                        et[:, :w], ps[:, :w],
                        mybir.ActivationFunctionType.Exp,
                        scale=0.125,
                    )
                    if j * P >= c * NC_CHUNK:
                        # zero the upper-right triangle of the diagonal
                        # 128-block (DVE is idle; keeps masking off ACT/PE)
                        nc.vector.tensor_mul(et[:, 0:P], et[:, 0:P], mskb)
                    # AV accumulate, per psum bank: bank b of this chunk
                    # has its last contribution at j == 8c + 4b + 3.
                    s0 = lo - c * NC_CHUNK
                    for b in range(NC_CHUNK // MM_N):
                        blo, bhi = b * MM_N, (b + 1) * MM_N
                        plo, phi = max(s0, blo), min(s0 + w, bhi)
                        if plo >= phi:
                            continue
                        nc.tensor.matmul(
                            av[0:DH + 1, ds(plo, phi - plo)],
                            lhsT=VA[:, j, h, :],
                            rhs=et[:, ds(plo - s0, phi - plo)],
                            start=(j == 0),
                            stop=(j == 8 * c + 4 * b + 3),
                        )
                # normalize: all off-ACT. Copy PSUM out first so the av
                # slot frees for the next head; reciprocal of the ones-row
                # on DVE (fast-approx Newton, ~18 bits -- plenty for a
                # bf16 multiply); broadcast on GpSimd; multiply on DVE.
                un = wnorm.tile([DH + 1, NC_CHUNK], F32, tag="un")
                nc.vector.tensor_copy(un, av[0:DH + 1, :])
                # engines can't shift partitions and partition_broadcast
                # reads physical partition 0, so DMA the denominator row
                # down to a partition-0 tile first; then 1/d on DVE
                # (fast-approx Newton, ~18 bits -- plenty ahead of a bf16
                # multiply) and broadcast across partitions on GpSimd.
                dr = wnorm.tile([1, NC_CHUNK], F32, tag="dr")
                nc.sync.dma_start(dr, un[DH:DH + 1, :])
                rc = wnorm.tile([1, NC_CHUNK], F32, tag="rc")
                nc.vector.reciprocal_approx_fast(rc, dr)
                rb = wnorm.tile([DH, NC_CHUNK], F32, tag="rb")
                nc.gpsimd.partition_broadcast(rb, rc)
                if sub == 0:
                    nc.vector.tensor_mul(
                        OGT[0:DH, hp, ds(c * NC_CHUNK, NC_CHUNK)],
                        un[0:DH, :], rb,
                    )
                else:
                    # DVE can't shift partitions; stage then DMA to 64:128
                    stg = wnorm.tile([DH, NC_CHUNK], BF16, tag="stg")
                    nc.vector.tensor_mul(stg, un[0:DH, :], rb)
                    nc.sync.dma_start(
                        OGT[DH:P, hp, ds(c * NC_CHUNK, NC_CHUNK)], stg,
                    )

            for h in range(NH):
                attn_head_chunk(h, 0)
            # chunk-0 OGT complete -> first-half o_proj can interleave
            # with the chunk-1 sweep
            for tt in range(8):
                for half in range(2):
                    add(("o", tt, half), make_oproj_fill(tt, half))
            for h in range(NH):
                attn_head_chunk(h, 1)
            pop_until(len(fills))  # drain any unpopped queue work
            for tt in range(8, NT):
                for half in range(2):
                    add(("o", tt, half), make_oproj_fill(tt, half))
            pop_until(len(fills))

    nc.compile()
    return nc


def _get_nc():
    global _CACHED
    if _CACHED is None:
        _CACHED = _build_kernel()
    return _CACHED


def _shard_inputs(x, wq, wk, wv, wo):
    bf = ml_dtypes.bfloat16
    in_maps = []
    for core in range(8):
        b, g = divmod(core, 2)
        gs = slice(g * DG, (g + 1) * DG)
        in_maps.append({
            "xT": np.ascontiguousarray(x[b].T).astype(bf),
            "wqT": np.ascontiguousarray(wq[gs, :].T).astype(bf),
            "wkT": np.ascontiguousarray(wk[gs, :].T).astype(bf),
            "wvT": np.ascontiguousarray(wv[gs, :].T).astype(bf),
            "woT": np.ascontiguousarray(wo[:, gs].T).astype(bf),
        })
    return in_maps


def kernel(x, wq, wk, wv, wo, _trace=False, _trace_cores=None):
    x = np.asarray(x, dtype=np.float32)
    wq = np.asarray(wq, dtype=np.float32)
    wk = np.asarray(wk, dtype=np.float32)
    wv = np.asarray(wv, dtype=np.float32)
    wo = np.asarray(wo, dtype=np.float32)

    nc = _get_nc()
    in_maps = _shard_inputs(x, wq, wk, wv, wo)
    res = run_bass_kernel_spmd(
        nc, in_maps, core_ids=list(range(8)),
        trace=_trace,
        **({"trace_cores": _trace_cores} if _trace_cores else {}),
    )
    B = x.shape[0]
    y = np.zeros((B, T, D), dtype=np.float32)
    for core in range(8):
        b = core // 2
        y[b] += res.results[core]["y"]
    if _trace:
        return y, res
    return y


# revision 12
# speedup vs baseline: 1.3968x; 1.0354x over previous
"""Causal self-attention kernel for Trainium2, sharded over 8 NeuronCores.

Problem: B=4, T=2048, DIM=1024, H=16 heads, head_dim=64, fp32 I/O.

Sharding: (batch, head-group) pairs -> 8 shards. Core c handles batch
b = c//2 and head group g = c%2 (8 heads each). Each core computes its
q/k/v projections for its head slice, causal flash-style attention, and
a partial o_proj against its head-slice of wo. The host sums the two
partial o_proj outputs per batch (the "all-reduce") while gathering.

Schedule strategy (per core): the attention inner loop is ScalarE-bound
(exp runs at 1.2 GHz x 1 elem/lane; the PE needs only ~60% of that
time for scores+AV), so all projection/o_proj matmuls are emitted as a
work queue of half-fills interleaved into the attention j-loop. The
Tile scheduler then keeps the PE dense (no HAM re-throttle) while ACT
streams exps. Heads run one at a time (not in pairs): that halves the
live PSUM set (scores 2x2 banks double-buffered + AV 2 banks) and
leaves 2 banks for the projection queue's accumulators.

Layout (per core):
  - Host pre-transposes x and the weight slices so the contraction dim
    lands on SBUF partitions, and casts to bf16.
  - Scores are computed TRANSPOSED: sT[tk, tq] = k @ q^T, so softmax'd
    probabilities come out with tk on partitions -- the layout the
    attn@v matmul needs as its moving operand (lhsT = v).
  - Softmax skips max-subtraction (scores are O(1) by construction),
    exp runs on ScalarE straight out of PSUM, and the denominator is
    free via a ones column appended to v.
  - 1/denominator runs on the (otherwise idle) DVE via the fast-approx
    Newton reciprocal, broadcast on GpSimd -- no ACT work at all, so
    ScalarE does nothing but exp.
  - Causal masking inside diagonal 128-tiles: one DVE multiply with a
    0/1 lower-triangle mask.
"""

import numpy as np
import ml_dtypes

import concourse.bass as bass
import concourse.bacc as bacc
import concourse.mybir as mybir
import concourse.tile as tile
from concourse.bass import ds, ts
from concourse.bass_utils import run_bass_kernel_spmd

BF16 = mybir.dt.bfloat16
F32 = mybir.dt.float32

T = 2048
D = 1024
DG = 512          # head-group width (8 heads x 64)
NH = 8            # heads per core
DH = 64
P = 128
NT = T // P       # 16 t-tiles
NKO = D // P      # 8 contraction tiles for projections
NC_CHUNK = 1024   # tq chunk width for attention
NCH = T // NC_CHUNK  # 2 chunks

_CACHED = None  # (nc, input names) -- build/trace once per process

MM_N = 512  # max moving free-dim per matmul instruction (one PSUM bank)


def _build_kernel():
    nc = bacc.Bacc("TRN2", target_bir_lowering=False, debug=False)

    xT_d = nc.dram_tensor("xT", [D, T], BF16, kind="ExternalInput").ap()
    wqT_d = nc.dram_tensor("wqT", [D, DG], BF16, kind="ExternalInput").ap()
    wkT_d = nc.dram_tensor("wkT", [D, DG], BF16, kind="ExternalInput").ap()
    wvT_d = nc.dram_tensor("wvT", [D, DG], BF16, kind="ExternalInput").ap()
    woT_d = nc.dram_tensor("woT", [DG, D], BF16, kind="ExternalInput").ap()
    y_d = nc.dram_tensor("y", [T, D], F32, kind="ExternalOutput").ap()

    with tile.TileContext(nc) as tc:
        with (
            tc.tile_pool(name="const", bufs=1) as const,
            tc.tile_pool(name="sb", bufs=1) as sb,
            tc.tile_pool(name="work", bufs=4) as work,
            tc.tile_pool(name="wnorm", bufs=2) as wnorm,
            tc.tile_pool(name="sc", bufs=2, space="PSUM") as scp,
            tc.tile_pool(name="av", bufs=1, space="PSUM") as avp,
            tc.tile_pool(name="pj", bufs=2, space="PSUM") as pjp,
        ):
            # ---- constants ----
            # multiplicative causal mask for diag tiles: 1 where tq >= tk
            mskb = const.tile([P, P], BF16, tag="mskb")
            nc.gpsimd.memset(mskb, 1.0)
            nc.gpsimd.affine_select(
                out=mskb, in_=mskb,
                compare_op=mybir.AluOpType.is_ge,
                fill=0.0, base=0,
                pattern=[[1, P]], channel_multiplier=-1,
            )

            # ---- persistent SBUF tensors ----
            XT = sb.tile([P, NKO, T], BF16, tag="XT")
            WQT = sb.tile([P, NKO, DG], BF16, tag="WQT")
            WKT = sb.tile([P, NKO, DG], BF16, tag="WKT")
            WVT = sb.tile([P, NKO, DG], BF16, tag="WVT")
            WOT = sb.tile([P, DG // P, D], BF16, tag="WOT")
            QT = sb.tile([P, DG // P, T], BF16, tag="QT")
            KT = sb.tile([P, DG // P, T], BF16, tag="KT")
            # half-swapped copies (partitions 0:64 <-> 64:128) so a lone
            # head can run two adjacent j-tiles' score matmuls on
            # disjoint PE row groups concurrently
            QTs = sb.tile([P, DG // P, T], BF16, tag="QTs")
            KTs = sb.tile([P, DG // P, T], BF16, tag="KTs")
            VA = sb.tile([P, NT, NH, DH + 1], BF16, tag="VA")
            OGT = sb.tile([P, DG // P, T], BF16, tag="OGT")

            # ---- input DMAs, ordered so the first head's prerequisites
            # (x tq 0:1024, q/k weights for dg 0, all of wv) land first ----
            xr = xT_d.rearrange("(ko p) t -> p ko t", p=P)
            wqr = wqT_d.rearrange("(ko p) n -> p ko n", p=P)
            wkr = wkT_d.rearrange("(ko p) n -> p ko n", p=P)
            for tcn in range(2):
                for k in range(NKO):
                    nc.sync.dma_start(
                        XT[:, k, ds(tcn * MM_N, MM_N)],
                        xr[:, k, ds(tcn * MM_N, MM_N)],
                    )
            for wsb, wr in ((WQT, wqr), (WKT, wkr)):
                for k in range(NKO):
                    nc.sync.dma_start(wsb[:, k, 0:P], wr[:, k, 0:P])
            wvr = wvT_d.rearrange("(ko p) n -> p ko n", p=P)
            for k in range(NKO):
                nc.sync.dma_start(WVT[:, k, :], wvr[:, k, :])
            for tcn in range(2, T // MM_N):
                for k in range(NKO):
                    nc.sync.dma_start(
                        XT[:, k, ds(tcn * MM_N, MM_N)],
                        xr[:, k, ds(tcn * MM_N, MM_N)],
                    )
            for wsb, wr in ((WQT, wqr), (WKT, wkr)):
                for k in range(NKO):
                    nc.sync.dma_start(wsb[:, k, ds(P, DG - P)], wr[:, k, ds(P, DG - P)])
            wor = woT_d.rearrange("(jo p) n -> p jo n", p=P)
            for j in range(DG // P):
                nc.sync.dma_start(WOT[:, j, :], wor[:, j, :])

            # v_aug ones column
            nc.gpsimd.memset(VA[:, :, :, DH], 1.0)

            # ---- projection work queue (half-fill closures) ----
            # Each fill accumulates 1 PSUM bank over its contraction and
            # is split into two emission halves so PE bursts stay under
            # ~1us and never starve ACT of its next scores tile.
            def make_qk_fill(wsb, dst, dsts, dg, tcn):
                st = {}

                def h1():
                    ps = pjp.tile([P, MM_N], F32, tag="pj")
                    st["ps"] = ps
                    for k in range(4):
                        nc.tensor.matmul(
                            ps, lhsT=wsb[:, k, ts(dg, P)],
                            rhs=XT[:, k, ds(tcn * MM_N, MM_N)],
                            start=(k == 0), stop=False,
                        )

                def h2():
                    ps = st["ps"]
                    for k in range(4, NKO):
                        nc.tensor.matmul(
                            ps, lhsT=wsb[:, k, ts(dg, P)],
                            rhs=XT[:, k, ds(tcn * MM_N, MM_N)],
                            start=False, stop=(k == NKO - 1),
                        )
                    sl = ds(tcn * MM_N, MM_N)
                    nc.vector.tensor_copy(dst[:, dg, sl], ps)
                    # build the half-swapped copy (partition shift via DMA)
                    nc.sync.dma_start(dsts[0:DH, dg, sl], dst[DH:P, dg, sl])
                    nc.sync.dma_start(dsts[DH:P, dg, sl], dst[0:DH, dg, sl])

                return [h1, h2]

            def make_v_fill(tt):
                st = {}

                def h1():
                    ps = pjp.tile([P, MM_N], F32, tag="pj")
                    st["ps"] = ps
                    for k in range(4):
                        nc.tensor.matmul(
                            ps, lhsT=XT[:, k, ts(tt, P)], rhs=WVT[:, k, :],
                            start=(k == 0), stop=False,
                        )

                def h2():
                    ps = st["ps"]
                    for k in range(4, NKO):
                        nc.tensor.matmul(
                            ps, lhsT=XT[:, k, ts(tt, P)], rhs=WVT[:, k, :],
                            start=False, stop=(k == NKO - 1),
                        )
                    nc.vector.tensor_copy(
                        VA[:, tt, :, 0:DH],
                        ps.rearrange("p (h d) -> p h d", h=NH),
                    )

                return [h1, h2]

            def make_oproj_fill(tt, half):
                st = {}

                def h1():
                    ps = pjp.tile([P, MM_N], F32, tag="pj")
                    st["ps"] = ps
                    for jt in range(2):
                        nc.tensor.matmul(
                            ps, lhsT=OGT[:, jt, ts(tt, P)],
                            rhs=WOT[:, jt, ds(half * MM_N, MM_N)],
                            start=(jt == 0), stop=False,
                        )

                def h2():
                    ps = st["ps"]
                    for jt in range(2, DG // P):
                        nc.tensor.matmul(
                            ps, lhsT=OGT[:, jt, ts(tt, P)],
                            rhs=WOT[:, jt, ds(half * MM_N, MM_N)],
                            start=False, stop=(jt == DG // P - 1),
                        )
                    ysb = wnorm.tile([P, MM_N], F32, tag="ysb")
                    nc.vector.tensor_copy(ysb, ps)
                    nc.sync.dma_start(y_d[ts(tt, P), ds(half * MM_N, MM_N)], ysb)

                return [h1, h2]

            # Queue order: everything chunk-0 attention needs first (all
            # pairs' q/k for tq 0:1024, v tiles 0..7), then the chunk-1
            # prerequisites. o_proj halves are appended between sweeps.
            # fill_end[key] = queue index at which that tensor region is
            # fully emitted, so the attention loop can pull exactly its
            # prerequisites and otherwise drain at a steady 1 half per j.
            fills = []
            fill_end = {}

            def add(key, halves):
                fills.extend(halves)
                fill_end[key] = len(fills)

            add(("q", 0, 0), make_qk_fill(WQT, QT, QTs, 0, 0))
            add(("k", 0, 0), make_qk_fill(WKT, KT, KTs, 0, 0))
            add(("q", 0, 1), make_qk_fill(WQT, QT, QTs, 0, 1))
            add(("v", 0), make_v_fill(0))
            add(("k", 0, 1), make_qk_fill(WKT, KT, KTs, 0, 1))
            add(("v", 1), make_v_fill(1))
            for tt in range(2, 8):
                add(("v", tt), make_v_fill(tt))
            for dg in range(1, 4):
                for tcn in range(2):
                    add(("q", dg, tcn), make_qk_fill(WQT, QT, QTs, dg, tcn))
                    add(("k", dg, tcn), make_qk_fill(WKT, KT, KTs, dg, tcn))
            for tcn in range(2, 4):
                add(("q", 0, tcn), make_qk_fill(WQT, QT, QTs, 0, tcn))
                add(("k", 0, tcn), make_qk_fill(WKT, KT, KTs, 0, tcn))
            for tt in range(8, NT):
                add(("v", tt), make_v_fill(tt))
            for dg in range(1, 4):
                for tcn in range(2, 4):
                    add(("q", dg, tcn), make_qk_fill(WQT, QT, QTs, dg, tcn))
                    add(("k", dg, tcn), make_qk_fill(WKT, KT, KTs, dg, tcn))

            state = {"fi": 0}

            def pop_until(idx):
                while state["fi"] < idx:
                    fills[state["fi"]]()
                    state["fi"] += 1

            def pop(n=1):
                pop_until(min(state["fi"] + n, len(fills)))

            def need(keys):
                pop_until(max(fill_end[k] for k in keys))

            # ---- attention: chunk-major sweep over heads ----
            def attn_head_chunk(h, c):
                hp, sub = divmod(h, 2)
                po = sub * DH
                av = avp.tile([DH + 1, NC_CHUNK], F32, tag="av")
                jmax = (c + 1) * NC_CHUNK // P - 1
                for jp in range(0, jmax + 1, 2):
                    js = [j for j in (jp, jp + 1) if j <= jmax]
                    req = [("q", hp, tcn)
                           for tcn in range(max(c * NC_CHUNK, jp * P) // MM_N,
                                            2 * (c + 1))]
                    for j in js:
                        req += [("k", hp, j * P // MM_N), ("v", j)]
                    need(req)
                    pop(len(js))
                    # scores for j and j+1 interleaved: j uses this head's
                    # native partition half, j+1 the swapped copy, so the
                    # two streams occupy disjoint PE row groups and run
                    # concurrently in the array.
                    tiles = []
                    for j in js:
                        lo = max(c * NC_CHUNK, j * P)
                        w = (c + 1) * NC_CHUNK - lo
                        ps = scp.tile([P, NC_CHUNK], F32, tag="sc")
                        tiles.append([j, lo, w, ps, 0])
                    while any(t[4] < t[2] for t in tiles):
                        for i, t in enumerate(tiles):
                            j, lo, w, ps, o = t
                            if o >= w:
                                continue
                            ww = min(w - o, MM_N)
                            if i == 0:
                                kt, qt, base = KT, QT, po
                            else:
                                kt, qt, base = KTs, QTs, DH - po
                            nc.tensor.matmul(
                                ps[:, ds(o, ww)],
                                lhsT=kt[base:base + DH, hp, ts(j, P)],
                                rhs=qt[base:base + DH, hp, ds(lo + o, ww)],
                                start=True, stop=True,
                            )
                            t[4] = o + ww
                    for j, lo, w, ps, _ in tiles:
                        et = work.tile([P, NC_CHUNK], BF16, tag="et")
                        nc.scalar.activation(
                            et[:, :w], ps[:, :w],
                            mybir.ActivationFunctionType.Exp,
                            scale=0.125,
                        )
                        if j * P >= c * NC_CHUNK:
                            # zero below the diagonal of the diag 128-block
                            # (DVE is idle; keeps masking off ACT/PE)
                            nc.vector.tensor_mul(et[:, 0:P], et[:, 0:P], mskb)
                        # AV accumulate, per psum bank: bank b of this chunk
                        # has its last contribution at j == 8c + 4b + 3.
                        s0 = lo - c * NC_CHUNK
                        for b in range(NC_CHUNK // MM_N):
                            blo, bhi = b * MM_N, (b + 1) * MM_N
                            plo, phi = max(s0, blo), min(s0 + w, bhi)
                            if plo >= phi:
                                continue
                            nc.tensor.matmul(
                                av[0:DH + 1, ds(plo, phi - plo)],
                                lhsT=VA[:, j, h, :],
                                rhs=et[:, ds(plo - s0, phi - plo)],
                                start=(j == 0),
                                stop=(j == 8 * c + 4 * b + 3),
                            )
                # normalize: all off-ACT. Copy PSUM out first so the av
                # slot frees for the next head; reciprocal of the ones-row
                # on DVE (fast-approx Newton, ~18 bits -- plenty for a
                # bf16 multiply); broadcast on GpSimd; multiply on DVE.
                un = wnorm.tile([DH + 1, NC_CHUNK], F32, tag="un")
                nc.vector.tensor_copy(un, av[0:DH + 1, :])
                # engines can't shift partitions and partition_broadcast
                # reads physical partition 0, so DMA the denominator row
                # down to a partition-0 tile first; then 1/d on DVE
                # (fast-approx Newton, ~18 bits -- plenty ahead of a bf16
                # multiply) and broadcast across partitions on GpSimd.
                dr = wnorm.tile([1, NC_CHUNK], F32, tag="dr")
                nc.sync.dma_start(dr, un[DH:DH + 1, :])
                rc = wnorm.tile([1, NC_CHUNK], F32, tag="dr")
                nc.vector.reciprocal_approx_fast(rc, dr)
                rb = wnorm.tile([DH, NC_CHUNK], F32, tag="rb")
                nc.gpsimd.partition_broadcast(rb, rc)
                if sub == 0:
                    nc.vector.tensor_mul(
                        OGT[0:DH, hp, ds(c * NC_CHUNK, NC_CHUNK)],
                        un[0:DH, :], rb,
                    )
                else:
                    # DVE can't shift partitions; stage then DMA to 64:128
                    stg = wnorm.tile([DH, NC_CHUNK], BF16, tag="stg")
                    nc.vector.tensor_mul(stg, un[0:DH, :], rb)
                    nc.sync.dma_start(
                        OGT[DH:P, hp, ds(c * NC_CHUNK, NC_CHUNK)], stg,
                    )

            for h in range(NH):
                attn_head_chunk(h, 0)
            # chunk-0 OGT complete -> first-half o_proj can interleave
            # with the chunk-1 sweep
            for tt in range(8):
                for half in range(2):
                    add(("o", tt, half), make_oproj_fill(tt, half))
            for h in range(NH):
                attn_head_chunk(h, 1)
            pop_until(len(fills))  # drain any unpopped queue work
            for tt in range(8, NT):
                for half in range(2):
                    add(("o", tt, half), make_oproj_fill(tt, half))
            pop_until(len(fills))

    nc.compile()
    return nc


def _get_nc():
    global _CACHED
    if _CACHED is None:
        _CACHED = _build_kernel()
    return _CACHED


def _shard_inputs(x, wq, wk, wv, wo):
    bf = ml_dtypes.bfloat16
    in_maps = []
    for core in range(8):
        b, g = divmod(core, 2)
        gs = slice(g * DG, (g + 1) * DG)
        in_maps.append({
            "xT": np.ascontiguousarray(x[b].T).astype(bf),
            "wqT": np.ascontiguousarray(wq[gs, :].T).astype(bf),
            "wkT": np.ascontiguousarray(wk[gs, :].T).astype(bf),
            "wvT": np.ascontiguousarray(wv[gs, :].T).astype(bf),
            "woT": np.ascontiguousarray(wo[:, gs].T).astype(bf),
        })
    return in_maps


def kernel(x, wq, wk, wv, wo, _trace=False, _trace_cores=None):
    x = np.asarray(x, dtype=np.float32)
    wq = np.asarray(wq, dtype=np.float32)
    wk = np.asarray(wk, dtype=np.float32)
    wv = np.asarray(wv, dtype=np.float32)
    wo = np.asarray(wo, dtype=np.float32)

    nc = _get_nc()
    in_maps = _shard_inputs(x, wq, wk, wv, wo)
    res = run_bass_kernel_spmd(
        nc, in_maps, core_ids=list(range(8)),
        trace=_trace,
        **({"trace_cores": _trace_cores} if _trace_cores else {}),
    )
    B = x.shape[0]
    y = np.zeros((B, T, D), dtype=np.float32)
    for core in range(8):
        b = core // 2
        y[b] += res.results[core]["y"]
    if _trace:
        return y, res
    return y


# revision 18
# speedup vs baseline: 1.4098x; 1.0093x over previous
"""Causal self-attention kernel for Trainium2, sharded over 8 NeuronCores.

Problem: B=4, T=2048, DIM=1024, H=16 heads, head_dim=64, fp32 I/O.

Sharding: (batch, head-group) pairs -> 8 shards. Core c handles batch
b = c//2 and head group g = c%2 (8 heads each). Each core computes its
q/k/v projections for its head slice, causal flash-style attention, and
a partial o_proj against its head-slice of wo. The host sums the two
partial o_proj outputs per batch (the "all-reduce") while gathering.

Schedule strategy (per core): the attention inner loop is ScalarE-bound
(exp runs at 1.2 GHz x 1 elem/lane; the PE needs only ~60% of that
time for scores+AV), so all projection/o_proj matmuls are emitted as a
work queue of half-fills interleaved into the attention j-loop. The
Tile scheduler then keeps the PE dense (no HAM re-throttle) while ACT
streams exps. Heads run one at a time (not in pairs): that halves the
live PSUM set (scores 2x2 banks double-buffered + AV 2 banks) and
leaves 2 banks for the projection queue's accumulators.

Layout (per core):
  - Host pre-transposes x and the weight slices so the contraction dim
    lands on SBUF partitions, and casts to bf16.
  - Scores are computed TRANSPOSED: sT[tk, tq] = k @ q^T, so softmax'd
    probabilities come out with tk on partitions -- the layout the
    attn@v matmul needs as its moving operand (lhsT = v).
  - Softmax skips max-subtraction (scores are O(1) by construction),
    exp runs on ScalarE straight out of PSUM, and the denominator is
    free via a ones column appended to v.
  - 1/denominator runs on the (otherwise idle) DVE via the fast-approx
    Newton reciprocal, broadcast on GpSimd -- no ACT work at all, so
    ScalarE does nothing but exp.
  - Causal masking inside diagonal 128-tiles: one DVE multiply with a
    0/1 lower-triangle mask.
"""

import numpy as np
import ml_dtypes

import concourse.bass as bass
import concourse.bacc as bacc
import concourse.mybir as mybir
import concourse.tile as tile
from concourse.bass import ds, ts
from concourse.bass_utils import run_bass_kernel_spmd

BF16 = mybir.dt.bfloat16
F32 = mybir.dt.float32

T = 2048
D = 1024
DG = 512          # head-group width (8 heads x 64)
NH = 8            # heads per core
DH = 64
P = 128
NT = T // P       # 16 t-tiles
NKO = D // P      # 8 contraction tiles for projections
NC_CHUNK = 1024   # tq chunk width for attention
NCH = T // NC_CHUNK  # 2 chunks

_CACHED = None  # (nc, input names) -- build/trace once per process

MM_N = 512  # max moving free-dim per matmul instruction (one PSUM bank)


def _build_kernel():
    nc = bacc.Bacc("TRN2", target_bir_lowering=False, debug=False)

    xT_d = nc.dram_tensor("xT", [D, T], BF16, kind="ExternalInput").ap()
    wqT_d = nc.dram_tensor("wqT", [D, DG], BF16, kind="ExternalInput").ap()
    wkT_d = nc.dram_tensor("wkT", [D, DG], BF16, kind="ExternalInput").ap()
    wvT_d = nc.dram_tensor("wvT", [D, DG], BF16, kind="ExternalInput").ap()
    woT_d = nc.dram_tensor("woT", [DG, D], BF16, kind="ExternalInput").ap()
    y_d = nc.dram_tensor("y", [T, D], F32, kind="ExternalOutput").ap()

    with tile.TileContext(nc) as tc:
        with (
            tc.tile_pool(name="const", bufs=1) as const,
            tc.tile_pool(name="sb", bufs=1) as sb,
            tc.tile_pool(name="work", bufs=4) as work,
            tc.tile_pool(name="wnorm", bufs=2) as wnorm,
            tc.tile_pool(name="sc", bufs=2, space="PSUM") as scp,
            tc.tile_pool(name="av", bufs=1, space="PSUM") as avp,
            tc.tile_pool(name="pj", bufs=2, space="PSUM") as pjp,
        ):
            # ---- constants ----
            # multiplicative causal mask for diag tiles: 1 where tq >= tk
            mskb = const.tile([P, P], BF16, tag="mskb")
            nc.gpsimd.memset(mskb, 1.0)
            nc.gpsimd.affine_select(
                out=mskb, in_=mskb,
                compare_op=mybir.AluOpType.is_ge,
                fill=0.0, base=0,
                pattern=[[1, P]], channel_multiplier=-1,
            )

            # ---- persistent SBUF tensors ----
            XT = sb.tile([P, NKO, T], BF16, tag="XT")
            WQT = sb.tile([P, NKO, DG], BF16, tag="WQT")
            WKT = sb.tile([P, NKO, DG], BF16, tag="WKT")
            WVT = sb.tile([P, NKO, DG], BF16, tag="WVT")
            WOT = sb.tile([P, DG // P, D], BF16, tag="WOT")
            QT = sb.tile([P, DG // P, T], BF16, tag="QT")
            KT = sb.tile([P, DG // P, T], BF16, tag="KT")
            # half-swapped copies (partitions 0:64 <-> 64:128) so a lone
            # head can run two adjacent j-tiles' score matmuls on
            # disjoint PE row groups concurrently
            QTs = sb.tile([P, DG // P, T], BF16, tag="QTs")
            KTs = sb.tile([P, DG // P, T], BF16, tag="KTs")
            VA = sb.tile([P, NT, NH, DH + 1], BF16, tag="VA")
            OGT = sb.tile([P, DG // P, T], BF16, tag="OGT")

            # ---- input DMAs: only the first head's prerequisites go up
            # front (x tq 0:1024, q/k weights for dg 0, all of wv); the
            # rest are paced through the work queue so latency-critical
            # small DMAs (half-swap copies, denominator rows) don't sit
            # behind megabytes of bulk input in the HWDGE queues ----
            xr = xT_d.rearrange("(ko p) t -> p ko t", p=P)
            wqr = wqT_d.rearrange("(ko p) n -> p ko n", p=P)
            wkr = wkT_d.rearrange("(ko p) n -> p ko n", p=P)
            for tcn in range(2):
                for k in range(NKO):
                    nc.sync.dma_start(
                        XT[:, k, ds(tcn * MM_N, MM_N)],
                        xr[:, k, ds(tcn * MM_N, MM_N)],
                    )
            for wsb, wr in ((WQT, wqr), (WKT, wkr)):
                for k in range(NKO):
                    nc.sync.dma_start(wsb[:, k, 0:P], wr[:, k, 0:P])
            wvr = wvT_d.rearrange("(ko p) n -> p ko n", p=P)
            for k in range(NKO):
                nc.sync.dma_start(WVT[:, k, :], wvr[:, k, :])

            def xt_dma(tcn):
                def go():
                    for k in range(NKO):
                        nc.sync.dma_start(
                            XT[:, k, ds(tcn * MM_N, MM_N)],
                            xr[:, k, ds(tcn * MM_N, MM_N)],
                        )
                return [go]

            def wrest_dma(wsb, wr):
                def go():
                    for k in range(NKO):
                        nc.sync.dma_start(
                            wsb[:, k, ds(P, DG - P)], wr[:, k, ds(P, DG - P)],
                        )
                return [go]

            def wo_dma():
                wor = woT_d.rearrange("(jo p) n -> p jo n", p=P)

                def go():
                    for j in range(DG // P):
                        nc.sync.dma_start(WOT[:, j, :], wor[:, j, :])
                return [go]

            # v_aug ones column
            nc.gpsimd.memset(VA[:, :, :, DH], 1.0)

            # ---- projection work queue (half-fill closures) ----
            # Each fill accumulates 1 PSUM bank over its contraction and
            # is split into two emission halves so PE bursts stay under
            # ~1us and never starve ACT of its next scores tile.
            def make_qk_fill(wsb, dst, dsts, dg, tcn):
                st = {}

                def h1():
                    ps = pjp.tile([P, MM_N], F32, tag="pj")
                    st["ps"] = ps
                    for k in range(4):
                        nc.tensor.matmul(
                            ps, lhsT=wsb[:, k, ts(dg, P)],
                            rhs=XT[:, k, ds(tcn * MM_N, MM_N)],
                            start=(k == 0), stop=False,
                        )

                def h2():
                    ps = st["ps"]
                    for k in range(4, NKO):
                        nc.tensor.matmul(
                            ps, lhsT=wsb[:, k, ts(dg, P)],
                            rhs=XT[:, k, ds(tcn * MM_N, MM_N)],
                            start=False, stop=(k == NKO - 1),
                        )
                    sl = ds(tcn * MM_N, MM_N)
                    nc.vector.tensor_copy(dst[:, dg, sl], ps)
                    # build the half-swapped copy (partition shift via DMA)
                    nc.sync.dma_start(dsts[0:DH, dg, sl], dst[DH:P, dg, sl])
                    nc.sync.dma_start(dsts[DH:P, dg, sl], dst[0:DH, dg, sl])

                return [h1, h2]

            def make_v_fill(tt):
                st = {}

                def h1():
                    ps = pjp.tile([P, MM_N], F32, tag="pj")
                    st["ps"] = ps
                    for k in range(4):
                        nc.tensor.matmul(
                            ps, lhsT=XT[:, k, ts(tt, P)], rhs=WVT[:, k, :],
                            start=(k == 0), stop=False,
                        )

                def h2():
                    ps = st["ps"]
                    for k in range(4, NKO):
                        nc.tensor.matmul(
                            ps, lhsT=XT[:, k, ts(tt, P)], rhs=WVT[:, k, :],
                            start=False, stop=(k == NKO - 1),
                        )
                    nc.vector.tensor_copy(
                        VA[:, tt, :, 0:DH],
                        ps.rearrange("p (h d) -> p h d", h=NH),
                    )

                return [h1, h2]

            def make_oproj_fill(tt, half, pool=None):
                st = {}

                def h1():
                    pl = pool if pool is not None else pjp
                    ps = pl.tile([P, MM_N], F32,
                                 tag="pj" if pl is pjp else "sc")
                    st["ps"] = ps
                    for jt in range(2):
                        nc.tensor.matmul(
                            ps, lhsT=OGT[:, jt, ts(tt, P)],
                            rhs=WOT[:, jt, ds(half * MM_N, MM_N)],
                            start=(jt == 0), stop=False,
                        )

                def h2():
                    ps = st["ps"]
                    for jt in range(2, DG // P):
                        nc.tensor.matmul(
                            ps, lhsT=OGT[:, jt, ts(tt, P)],
                            rhs=WOT[:, jt, ds(half * MM_N, MM_N)],
                            start=False, stop=(jt == DG // P - 1),
                        )
                    ysb = wnorm.tile([P, MM_N], F32, tag="ysb")
                    nc.vector.tensor_copy(ysb, ps)
                    nc.sync.dma_start(y_d[ts(tt, P), ds(half * MM_N, MM_N)], ysb)

                return [h1, h2]

            # Queue order: everything chunk-0 attention needs first (all
            # pairs' q/k for tq 0:1024, v tiles 0..7), then the chunk-1
            # prerequisites. o_proj halves are appended between sweeps.
            # fill_end[key] = queue index at which that tensor region is
            # fully emitted, so the attention loop can pull exactly its
            # prerequisites and otherwise drain at a steady 1 half per j.
            fills = []
            fill_end = {}

            def add(key, halves):
                fills.extend(halves)
                fill_end[key] = len(fills)

            add(("q", 0, 0), make_qk_fill(WQT, QT, QTs, 0, 0))
            add(("k", 0, 0), make_qk_fill(WKT, KT, KTs, 0, 0))
            add(("q", 0, 1), make_qk_fill(WQT, QT, QTs, 0, 1))
            add(("v", 0), make_v_fill(0))
            add(("k", 0, 1), make_qk_fill(WKT, KT, KTs, 0, 1))
            add(("v", 1), make_v_fill(1))
            add(("wq_rest",), wrest_dma(WQT, wqr))
            add(("wk_rest",), wrest_dma(WKT, wkr))
            for tt in range(2, 8):
                add(("v", tt), make_v_fill(tt))
            for dg in range(1, 4):
                for tcn in range(2):
                    add(("q", dg, tcn), make_qk_fill(WQT, QT, QTs, dg, tcn))
                    add(("k", dg, tcn), make_qk_fill(WKT, KT, KTs, dg, tcn))
            add(("xt", 2), xt_dma(2))
            add(("xt", 3), xt_dma(3))
            add(("wo",), wo_dma())
            for tcn in range(2, 4):
                add(("q", 0, tcn), make_qk_fill(WQT, QT, QTs, 0, tcn))
                add(("k", 0, tcn), make_qk_fill(WKT, KT, KTs, 0, tcn))
            for tt in range(8, NT):
                add(("v", tt), make_v_fill(tt))
            for dg in range(1, 4):
                for tcn in range(2, 4):
                    add(("q", dg, tcn), make_qk_fill(WQT, QT, QTs, dg, tcn))
                    add(("k", dg, tcn), make_qk_fill(WKT, KT, KTs, dg, tcn))

            state = {"fi": 0, "hold": 0}

            def pop_until(idx):
                while state["fi"] < idx:
                    fills[state["fi"]]()
                    state["fi"] += 1

            def pop(n=1):
                # paced draining respects the hold-back reservation (work
                # kept for the final head's window); need() ignores it
                pop_until(min(state["fi"] + n, len(fills) - state["hold"]))

            def need(keys):
                pop_until(max(fill_end[k] for k in keys))

            # ---- attention: chunk-major sweep over heads ----
            def attn_head_chunk(h, c, pops=1):
                hp, sub = divmod(h, 2)
                po = sub * DH
                # the very first head-pair's chunk 0 runs while the PE is
                # still DMA-bound; skip pairing there so it doesn't wait
                # on the half-swap copies of QT/KT
                paired = not (hp == 0 and c == 0)
                step = 2 if paired else 1
                av = avp.tile([DH + 1, NC_CHUNK], F32, tag="av")
                jmax = (c + 1) * NC_CHUNK // P - 1
                for jp in range(0, jmax + 1, step):
                    js = [j for j in (jp, jp + step - 1) if j <= jmax][:step]
                    req = [("q", hp, tcn)
                           for tcn in range(max(c * NC_CHUNK, jp * P) // MM_N,
                                            2 * (c + 1))]
                    for j in js:
                        req += [("k", hp, j * P // MM_N), ("v", j)]
                    need(req)
                    pop(pops * len(js))
                    # scores for j and j+1 interleaved: j uses this head's
                    # native partition half, j+1 the swapped copy, so the
                    # two streams occupy disjoint PE row groups and run
                    # concurrently in the array.
                    tiles = []
                    for j in js:
                        lo = max(c * NC_CHUNK, j * P)
                        w = (c + 1) * NC_CHUNK - lo
                        ps = scp.tile([P, NC_CHUNK], F32, tag="sc")
                        tiles.append([j, lo, w, ps, 0])
                    while any(t[4] < t[2] for t in tiles):
                        for i, t in enumerate(tiles):
                            j, lo, w, ps, o = t
                            if o >= w:
                                continue
                            ww = min(w - o, MM_N)
                            if i == 0:
                                kt, qt, base = KT, QT, po
                            else:
                                kt, qt, base = KTs, QTs, DH - po
                            nc.tensor.matmul(
                                ps[:, ds(o, ww)],
                                lhsT=kt[base:base + DH, hp, ts(j, P)],
                                rhs=qt[base:base + DH, hp, ds(lo + o, ww)],
                                start=True, stop=True,
                            )
                            t[4] = o + ww
                    for j, lo, w, ps, _ in tiles:
                        et = work.tile([P, NC_CHUNK], BF16, tag="et")
                        nc.scalar.activation(
                            et[:, :w], ps[:, :w],
                            mybir.ActivationFunctionType.Exp,
                            scale=0.125,
                        )
                        if j * P >= c * NC_CHUNK:
                            # zero below the diagonal of the diag 128-block
                            # (DVE is idle; keeps masking off ACT/PE)
                            nc.vector.tensor_mul(et[:, 0:P], et[:, 0:P], mskb)
                        # AV accumulate, per psum bank: bank b of this chunk
                        # has its last contribution at j == 8c + 4b + 3.
                        s0 = lo - c * NC_CHUNK
                        for b in range(NC_CHUNK // MM_N):
                            blo, bhi = b * MM_N, (b + 1) * MM_N
                            plo, phi = max(s0, blo), min(s0 + w, bhi)
                            if plo >= phi:
                                continue
                            nc.tensor.matmul(
                                av[0:DH + 1, ds(plo, phi - plo)],
                                lhsT=VA[:, j, h, :],
                                rhs=et[:, ds(plo - s0, phi - plo)],
                                start=(j == 0),
                                stop=(j == 8 * c + 4 * b + 3),
                            )
                # normalize: all off-ACT. Copy PSUM out first so the av
                # slot frees for the next head; reciprocal of the ones-row
                # on DVE (fast-approx Newton, ~18 bits -- plenty for a
                # bf16 multiply); broadcast on GpSimd; multiply on DVE.
                un = wnorm.tile([DH + 1, NC_CHUNK], F32, tag="un")
                nc.vector.tensor_copy(un, av[0:DH + 1, :])
                # engines can't shift partitions and partition_broadcast
                # reads physical partition 0, so DMA the denominator row
                # down to a partition-0 tile first; then 1/d on DVE
                # (fast-approx Newton, ~18 bits -- plenty ahead of a bf16
                # multiply) and broadcast across partitions on GpSimd.
                dr = wnorm.tile([1, NC_CHUNK], F32, tag="dr")
                nc.sync.dma_start(dr, un[DH:DH + 1, :])
                rc = wnorm.tile([1, NC_CHUNK], F32, tag="dr")
                nc.vector.reciprocal_approx_fast(rc, dr)
                rb = wnorm.tile([DH, NC_CHUNK], F32, tag="rb")
                nc.gpsimd.partition_broadcast(rb, rc)
                if sub == 0:
                    nc.vector.tensor_mul(
                        OGT[0:DH, hp, ds(c * NC_CHUNK, NC_CHUNK)],
                        un[0:DH, :], rb,
                    )
                else:
                    # DVE can't shift partitions; stage then DMA to 64:128
                    stg = wnorm.tile([DH, NC_CHUNK], BF16, tag="stg")
                    nc.vector.tensor_mul(stg, un[0:DH, :], rb)
                    nc.sync.dma_start(
                        OGT[DH:P, hp, ds(c * NC_CHUNK, NC_CHUNK)], stg,
                    )

            for h in range(NH):
                attn_head_chunk(h, 0)
            # chunk-0 OGT complete -> first-half o_proj can interleave
            # with the chunk-1 sweep; reserve a slice of it for the last
            # head's window so the PE stays fed while the queue runs dry
            for tt in range(8):
                for half in range(2):
                    add(("o", tt, half), make_oproj_fill(tt, half))
            state["hold"] = 12
            for h in range(NH):
                if h == NH - 1:
                    state["hold"] = 0
                attn_head_chunk(h, 1)
            pop_until(len(fills))  # drain any unpopped queue work
            # pure-PE tail: alternate PSUM pools (the score pool is idle
            # now) so four o_proj accumulations pipeline
            for tt in range(8, NT):
                for half in range(2):
                    add(("o", tt, half),
                        make_oproj_fill(tt, half, scp if (tt + half) % 2 else pjp))
            pop_until(len(fills))

    nc.compile()
    return nc


def _get_nc():
    global _CACHED
    if _CACHED is None:
        _CACHED = _build_kernel()
    return _CACHED


def _shard_inputs(x, wq, wk, wv, wo):
    bf = ml_dtypes.bfloat16
    in_maps = []
    for core in range(8):
        b, g = divmod(core, 2)
        gs = slice(g * DG, (g + 1) * DG)
        in_maps.append({
            "xT": np.ascontiguousarray(x[b].T).astype(bf),
            "wqT": np.ascontiguousarray(wq[gs, :].T).astype(bf),
            "wkT": np.ascontiguousarray(wk[gs, :].T).astype(bf),
            "wvT": np.ascontiguousarray(wv[gs, :].T).astype(bf),
            "woT": np.ascontiguousarray(wo[:, gs].T).astype(bf),
        })
    return in_maps


def kernel(x, wq, wk, wv, wo, _trace=False, _trace_cores=None):
    x = np.asarray(x, dtype=np.float32)
    wq = np.asarray(wq, dtype=np.float32)
    wk = np.asarray(wk, dtype=np.float32)
    wv = np.asarray(wv, dtype=np.float32)
    wo = np.asarray(wo, dtype=np.float32)

    nc = _get_nc()
    in_maps = _shard_inputs(x, wq, wk, wv, wo)
    res = run_bass_kernel_spmd(
        nc, in_maps, core_ids=list(range(8)),
        trace=_trace,
        **({"trace_cores": _trace_cores} if _trace_cores else {}),
    )
    B = x.shape[0]
    y = np.zeros((B, T, D), dtype=np.float32)
    for core in range(8):
        b = core // 2
        y[b] += res.results[core]["y"]
    if _trace:
        return y, res
    return y
